# revision 5
# baseline (speedup 1.0000x reference)
"""Biased multi-head attention Trainium2 kernel (Bass/Tile), 8-way data-parallel over batch.

Reference computation (per batch b):
  q = (nd @ Wq + bq) * 8 ; k = nd @ Wk + bk ; v = nd @ Wv + bv      (8 heads, d=64)
  S[h] = Q_h K_h^T + bias[..,h] ; S[mask==1] = -inf
  A = softmax(S, -1) * mul[..,h]
  out = concat_h(A_h V_h) @ Wo + bo

Device mapping (per core, 2 batches):
  - host supplies nd pre-transposed (ndt [F, N] f32r); all fp32 matmuls run in
    float32r (full-rate PE streaming, ~1e-4 rel err)
  - QT/KT = W^T @ ndT with per-partition bias added via K=1 ones-row matmuls
  - scores: per (head-pair, i-chunk): K=64 f32r matmul + bias/mask chunk added
    in the same PSUM group via an id128 @ bias bf16 matmul
  - row max via one DVE tensor_reduce over [128, 2, 512]; exp on ACT straight
    from PSUM (bf16 out, Z via accumulator)
  - A = E*mul (gpsimd TT) then *1/Z (DVE tensor_scalar, 4x mode), bf16
  - A^T via DMA: pm chunks stored to DRAM scratch, transposed back with
    dma_start_transpose (DRAM->SBUF, exact)
  - AV accumulates per head pair in one PSUM tile (tile_position col split)
  - final projection OT bf16 @ Wo bf16 + bo ones-row matmul
"""

import os
import sys

import numpy as np

try:
    import concourse  # noqa: F401
except ImportError:
    sys.path.insert(0, "/opt/trn_rl_repo")

import ml_dtypes
from concourse import bass, mybir
from concourse.bass_utils import run_bass_kernel_spmd
from concourse.tile import TileContext

B, N, F, H, D = 16, 512, 512, 8, 64
NCORES = 8
BPC = B // NCORES  # batches per core
IC = N // 128      # 128-row chunks per sequence

f32 = mybir.dt.float32
f32r = mybir.dt.float32r
bf16 = mybir.dt.bfloat16
AF = mybir.ActivationFunctionType
OP = mybir.AluOpType
AX = mybir.AxisListType


def _split_multiwaits(nc: bass.Bass) -> bass.Bass:
    """Walrus codegen only accepts one sync-wait per ISA instruction; hoist
    extra waits into single-wait NoOps on the same engine right before."""
    for fn in nc.m.functions:
        for blk in fn.blocks:
            new = []
            for inst in blk.instructions:
                si = getattr(inst, "sync_info", None)
                ow = list(si.on_wait) if (si is not None and si.on_wait) else []
                if len(ow) > 1:
                    for j, w in enumerate(ow[:-1]):
                        new.append(mybir.InstNoOp(
                            name=f"{inst.name}-wsplit{j}",
                            engine=inst.engine,
                            ins=[], outs=[],
                            sync_info=mybir.SyncInfo(on_wait=[w], on_update=[]),
                        ))
                    si.on_wait = ow[-1:]
                    inst.sync_info = si
                new.append(inst)
            blk.instructions[:] = new
    return nc


def build_nc(split: bool = True) -> bass.Bass:
    nc = bass.Bass()

    ndt_d = nc.declare_dram_parameter("ndt", [BPC, F, N], f32r, isOutput=False)
    # bias with mask pre-merged (-1e30), layout [b, pair, ic, p, u, j] bf16
    bias_d = nc.declare_dram_parameter("bias", [BPC, 4, IC, 128, 2, N], bf16,
                                       isOutput=False)
    # attn_mul, same layout
    mul_d = nc.declare_dram_parameter("mul", [BPC, 4, IC, 128, 2, N], bf16,
                                      isOutput=False)
    wq_d = nc.declare_dram_parameter("wq", [F, F], f32r, isOutput=False)  # x8
    wk_d = nc.declare_dram_parameter("wk", [F, F], f32r, isOutput=False)
    wv_d = nc.declare_dram_parameter("wv", [F, F], f32r, isOutput=False)
    wo_d = nc.declare_dram_parameter("wo", [F, F], bf16, isOutput=False)
    # [bq*8 | bk | bv | bo | ones] as one row, f32r
    brow_d = nc.declare_dram_parameter("brow", [1, 5 * F], f32r, isOutput=False)
    idb_d = nc.declare_dram_parameter("idb", [128, 128], bf16, isOutput=False)
    out_d = nc.declare_dram_parameter("out", [BPC, N, F], f32, isOutput=True)

    with (
        TileContext(nc) as tc,
        tc.tile_pool(name="cpool", bufs=1) as cpool,
        tc.tile_pool(name="bpool", bufs=1) as bpool,
        tc.tile_pool(name="spool", bufs=2) as spool,
        tc.tile_pool(name="wpool", bufs=2) as wpool,
        tc.tile_pool(name="dpool", bufs=1, space="DRAM") as dpool,
        tc.tile_pool(name="ps_s", bufs=2, space="PSUM") as ps_s,
        tc.tile_pool(name="ps_mm", bufs=2, space="PSUM") as ps_mm,
        tc.tile_pool(name="ps_o", bufs=2, space="PSUM") as ps_o,
    ):
        # ---- constants / weights ----
        wq_sb = cpool.tile([128, 4 * F], f32r, name="wq_sb")
        wk_sb = cpool.tile([128, 4 * F], f32r, name="wk_sb")
        wv_sb = cpool.tile([128, 4 * F], f32r, name="wv_sb")
        wo_sb = cpool.tile([128, 4 * F], bf16, name="wo_sb")
        for cc in range(4):
            nc.scalar.dma_start(wq_sb[:, cc * F:(cc + 1) * F], wq_d[cc * 128:(cc + 1) * 128, :])
            nc.scalar.dma_start(wk_sb[:, cc * F:(cc + 1) * F], wk_d[cc * 128:(cc + 1) * 128, :])
        brow_sb = cpool.tile([1, 5 * F], f32r, name="brow_sb")
        nc.sync.dma_start(brow_sb[:], brow_d[:, :])
        bq_row = brow_sb[:, 0:F]            # [1, 512] per-ft chunks as lhsT
        bk_row = brow_sb[:, F:2 * F]
        bv_row = brow_sb[:, 2 * F:3 * F]    # rhs for V bias
        bo_row = brow_sb[:, 3 * F:4 * F]
        ones_row = brow_sb[:, 4 * F:5 * F]  # [1, 512] of ones
        idb_sb = cpool.tile([128, 128], bf16, name="idb_sb")
        nc.sync.dma_start(idb_sb[:], idb_d[:, :])

        def batch_inputs(b):
            st = {"b": b}
            ndt_sb = bpool.tile([128, 4 * N], f32r, name="ndt_sb", tag="ndt", bufs=2)
            for cc in range(4):
                nc.scalar.dma_start(ndt_sb[:, cc * N:(cc + 1) * N],
                                  ndt_d[b, cc * 128:(cc + 1) * 128, :])
            st["ndt"] = ndt_sb
            st["v"] = bpool.tile([128, 4 * F], bf16, name="v_sb", tag="v", bufs=2)
            st["ot"] = bpool.tile([128, 4 * N], bf16, name="ot_sb", tag="ot", bufs=2)
            if b == 0:
                for cc in range(4):
                    nc.scalar.dma_start(wv_sb[:, cc * F:(cc + 1) * F],
                                      wv_d[cc * 128:(cc + 1) * 128, :])
                    nc.scalar.dma_start(wo_sb[:, cc * F:(cc + 1) * F],
                                      wo_d[cc * 128:(cc + 1) * 128, :])
            return st

        def qk_proj(st, t):
            ndt_sb = st["ndt"]
            psq = ps_mm.tile([128, 512], f32, name="ps_q", tag="mm")
            for cc in range(4):
                nc.tensor.matmul(
                    psq[:],
                    lhsT=wq_sb[:, cc * F + t * 128: cc * F + t * 128 + 128],
                    rhs=ndt_sb[:, cc * N:(cc + 1) * N],
                    start=(cc == 0), stop=False,
                )
            nc.tensor.matmul(psq[:], lhsT=bq_row[:, t * 128:(t + 1) * 128],
                             rhs=ones_row, start=False, stop=True)
            qt = wpool.tile([128, 512], f32r, name="qt", tag="qt", bufs=3)
            nc.scalar.copy(qt[:], psq[:])
            psk = ps_mm.tile([128, 512], f32, name="ps_k", tag="mm")
            for cc in range(4):
                nc.tensor.matmul(
                    psk[:],
                    lhsT=wk_sb[:, cc * F + t * 128: cc * F + t * 128 + 128],
                    rhs=ndt_sb[:, cc * N:(cc + 1) * N],
                    start=(cc == 0), stop=False,
                )
            nc.tensor.matmul(psk[:], lhsT=bk_row[:, t * 128:(t + 1) * 128],
                             rhs=ones_row, start=False, stop=True)
            kt = wpool.tile([128, 512], f32r, name="kt", tag="kt", bufs=3)
            nc.vector.tensor_copy(kt[:], psk[:])
            st["qt"], st["kt"] = qt, kt

        def v_proj(st, jc):
            ndt_sb, v_sb = st["ndt"], st["v"]
            psv = ps_mm.tile([128, 512], f32, name="ps_v", tag="mm")
            for cc in range(4):
                nc.tensor.matmul(
                    psv[:],
                    lhsT=ndt_sb[:, cc * N + jc * 128: cc * N + jc * 128 + 128],
                    rhs=wv_sb[:, cc * F:(cc + 1) * F],
                    start=(cc == 0), stop=False,
                )
            nc.tensor.matmul(psv[:], lhsT=ones_row[:, 0:128], rhs=bv_row,
                             start=False, stop=True)
            nc.scalar.copy(v_sb[:, jc * F:(jc + 1) * F], psv[:])

        def prefetch_pair(b, t):
            """biasm/mul chunk DMAs for pair (b, t), issued one stage ahead."""
            tiles = []
            for ic in range(IC):
                biasic = spool.tile([128, 2, N], bf16, name="biasic",
                                    tag="biasic", bufs=9)
                nc.scalar.dma_start(biasic[:], bias_d[b, t, ic])
                mulic = spool.tile([128, 2, N], bf16, name="mulic",
                                   tag="mulic", bufs=9)
                nc.scalar.dma_start(mulic[:], mul_d[b, t, ic])
                tiles.append((biasic, mulic))
            return tiles

        def s_chunks(b, st, t, scr, tiles, ics, pend):
            """Scores + softmax for chunks `ics` of head pair t; pm chunks go
            to DRAM scr. Normalize is pipelined one chunk behind the exp."""
            qt, kt = st["qt"], st["kt"]

            def flush_pend():
                if pend[0] is None:
                    return
                ic, e_u, z2, mulic = pend[0]
                rz2 = wpool.tile([128, 2], f32, name="rz2", tag="rz2", bufs=4)
                nc.vector.reciprocal(rz2[:], z2[:])
                for u in range(2):
                    pm0 = wpool.tile([128, 512], bf16, name="pm0",
                                     tag="pm0", bufs=4)
                    nc.gpsimd.tensor_tensor(pm0[:], e_u[u][:], mulic[:, u, :],
                                            OP.mult)
                    pm = wpool.tile([128, 512], bf16, name="pm", tag="pm", bufs=6)
                    nc.vector.tensor_scalar(pm[:], pm0[:], rz2[:, u:u + 1],
                                            None, OP.mult)
                    nc.sync.dma_start(
                        scr[u, ic * 128:(ic + 1) * 128, :], pm[:])
                pend[0] = None

            for ic in ics:
                biasic, mulic = tiles[ic]
                z2 = wpool.tile([128, 2], f32, name="z2", tag="z2", bufs=4)
                e_u = []
                for u in range(2):
                    sp = ps_s.tile([128, N], f32, name="sp", tag="sp", bufs=4)
                    nc.tensor.matmul(
                        sp[:],
                        lhsT=qt[u * 64:u * 64 + 64, ic * 128:ic * 128 + 128],
                        rhs=kt[u * 64:u * 64 + 64, :],
                        start=True, stop=False,
                    )
                    nc.tensor.matmul(
                        sp[:], lhsT=idb_sb[:], rhs=biasic[:, u, :],
                        start=False, stop=True,
                    )
                    negmax = wpool.tile([128, 1], f32, name="negmax",
                                        tag="negmax", bufs=8)
                    nc.vector.tensor_reduce(negmax[:], sp[:], AX.X, OP.max,
                                            negate=True)
                    e_sb = wpool.tile([128, N], bf16, name="e_sb", tag="e",
                                      bufs=6)
                    nc.scalar.activation(e_sb[:], sp[:], AF.Exp,
                                         bias=negmax[:], scale=1.0,
                                         accum_out=z2[:, u:u + 1])
                    e_u.append(e_sb)
                flush_pend()
                pend[0] = (ic, e_u, z2, mulic)
            if not ics:
                flush_pend()

        def tav_stage(st, t, scr):
            """DMA-transpose pm from DRAM and run AV matmuls for head pair t."""
            v_sb, ot_sb = st["v"], st["ot"]
            av = ps_o.tile([128, N], f32, name="av_ps", tag="av")
            pmts = {}
            for jc in range(4):
                for u in range(2):
                    pmt = spool.tile([128, N], bf16, name="pmt", tag="pmt",
                                     bufs=10)
                    nc.sync.dma_start_transpose(
                        pmt[:], scr[u, :, jc * 128:(jc + 1) * 128])
                    pmts[jc, u] = pmt
            for jc in range(4):
                for u in range(2):
                    h = 2 * t + u
                    nc.tensor.matmul(
                        av[u * 64:(u + 1) * 64, :],
                        lhsT=v_sb[:, jc * F + h * 64: jc * F + h * 64 + 64],
                        rhs=pmts[jc, u][:],
                        start=(jc == 0), stop=(jc == 3),
                        tile_position=(0, u * 64),
                    )
            nc.vector.tensor_copy(ot_sb[:, t * N:(t + 1) * N], av[:])

        def final_stage(b, st):
            ot_sb = st["ot"]
            for icq in range(IC):
                f_ps = ps_mm.tile([128, N], f32, name="f_ps", tag="mm")
                for cc in range(4):
                    nc.tensor.matmul(
                        f_ps[:],
                        lhsT=ot_sb[:, cc * N + icq * 128: cc * N + icq * 128 + 128],
                        rhs=wo_sb[:, cc * F:(cc + 1) * F],
                        start=(cc == 0), stop=False,
                    )
                nc.tensor.matmul(f_ps[:], lhsT=ones_row[:, 0:128], rhs=bo_row,
                                 start=False, stop=True)
                out_sb = wpool.tile([128, N], f32, name="out_sb",
                                    tag="out", bufs=2)
                nc.scalar.copy(out_sb[:], f_ps[:])
                nc.sync.dma_start(out_d[b, icq * 128:(icq + 1) * 128, :], out_sb[:])

        # ---- cross-batch pair pipeline: 8 head-pair stages (2 batches x 4);
        # TAV of pair k-1 runs between the score chunk halves of pair k ----
        sts = {0: batch_inputs(0)}
        scrs = {}
        tiles = {0: prefetch_pair(0, 0)}
        for k in range(9):
            if k < 8:
                b, t = divmod(k, 4)
                st = sts[b]
                qk_proj(st, t)
                scrs[k] = dpool.tile([2, N, N], bf16, name="scr",
                                     tag="scr", bufs=3)
                pend = [None]
                s_chunks(b, st, t, scrs[k], tiles[k], [0, 1], pend)
                if k + 1 < 8:
                    bn, tn = divmod(k + 1, 4)
                    tiles[k + 1] = prefetch_pair(bn, tn)
            if k >= 1:
                bb, tt = divmod(k - 1, 4)
                tav_stage(sts[bb], tt, scrs[k - 1])
            if k < 8:
                s_chunks(b, st, t, scrs[k], tiles[k], [2, 3], pend)
            if k >= 1 and tt == 3:
                final_stage(bb, sts[bb])
            if k < 8:
                if t == 0:
                    for jc in range(4):
                        v_proj(st, jc)
                # flush the ic=3 normalize tail
                s_chunks(b, st, t, scrs[k], tiles[k], [], pend)
            if k == 2:
                sts[1] = batch_inputs(1)

    mybir.codegen_inst_isa_subclasses(nc)
    return _split_multiwaits(nc) if split else nc


def make_in_maps(inputs: dict) -> list:
    inp = {k: np.asarray(v) for k, v in inputs.items()}
    ident = np.eye(128, dtype=np.float32)
    brow = np.concatenate([
        inp["bq"].astype(np.float32) * 8.0,
        inp["bk"].astype(np.float32),
        inp["bv"].astype(np.float32),
        inp["bo"].astype(np.float32),
        np.ones(F, np.float32),
    ]).reshape(1, 5 * F)
    shared = {
        "wq": np.ascontiguousarray(inp["Wq"].astype(np.float32) * 8.0),
        "wk": np.ascontiguousarray(inp["Wk"].astype(np.float32)),
        "wv": np.ascontiguousarray(inp["Wv"].astype(np.float32)),
        "wo": np.ascontiguousarray(inp["Wo"].astype(ml_dtypes.bfloat16)),
        "brow": brow,
        "idb": ident.astype(ml_dtypes.bfloat16),
    }
    # [B, i, j, h] -> [B, pair, ic, p, u, j]
    def to_chunks(x):
        x = x.transpose(0, 3, 1, 2)                      # [B, h, i, j]
        x = x.reshape(B, 4, 2, IC, 128, N)               # [B, t, u, ic, p, j]
        return np.ascontiguousarray(x.transpose(0, 1, 3, 4, 2, 5))

    biasm = np.where(inp["attn_mask"][..., None] != 0, np.float32(-1e30),
                     inp["attn_bias"].astype(np.float32))
    biasm = to_chunks(biasm).astype(ml_dtypes.bfloat16)
    mul_t = to_chunks(inp["attn_mul"].astype(np.float32)).astype(ml_dtypes.bfloat16)
    ndt = np.ascontiguousarray(
        inp["ndata"].astype(np.float32).transpose(0, 2, 1))
    in_maps = []
    for c in range(NCORES):
        sl = slice(c * BPC, (c + 1) * BPC)
        m = dict(shared)
        m["ndt"] = np.ascontiguousarray(ndt[sl])
        m["bias"] = np.ascontiguousarray(biasm[sl])
        m["mul"] = np.ascontiguousarray(mul_t[sl])
        in_maps.append(m)
    return in_maps


def kernel(**inputs) -> np.ndarray:
    nc = build_nc()
    in_maps = make_in_maps(inputs)
    res = run_bass_kernel_spmd(nc, in_maps, list(range(NCORES)))
    out = np.concatenate([np.asarray(res.results[c]["out"]) for c in range(NCORES)],
                         axis=0)
    return out.astype(np.float32)


if __name__ == "__main__":
    nc = build_nc()
    print("built ok")


# revision 7
# speedup vs baseline: 1.0274x; 1.0274x over previous
"""Biased multi-head attention Trainium2 kernel (Bass/Tile), 8-way data-parallel over batch.

Reference computation (per batch b):
  q = (nd @ Wq + bq) * 8 ; k = nd @ Wk + bk ; v = nd @ Wv + bv      (8 heads, d=64)
  S[h] = Q_h K_h^T + bias[..,h] ; S[mask==1] = -inf
  A = softmax(S, -1) * mul[..,h]
  out = concat_h(A_h V_h) @ Wo + bo

Device mapping (per core, 2 batches):
  - host supplies nd pre-transposed (ndt [F, N] f32r); all fp32 matmuls run in
    float32r (full-rate PE streaming, ~1e-4 rel err)
  - QT/KT = W^T @ ndT with per-partition bias added via K=1 ones-row matmuls
  - scores: per (head-pair, i-chunk): K=64 f32r matmul + bias/mask chunk added
    in the same PSUM group via an id128 @ bias bf16 matmul
  - row max via one DVE tensor_reduce over [128, 2, 512]; exp on ACT straight
    from PSUM (bf16 out, Z via accumulator)
  - A = E*mul (gpsimd TT) then *1/Z (DVE tensor_scalar, 4x mode), bf16
  - A^T via DMA: pm chunks stored to DRAM scratch, transposed back with
    dma_start_transpose (DRAM->SBUF, exact)
  - AV accumulates per head pair in one PSUM tile (tile_position col split)
  - final projection OT bf16 @ Wo bf16 + bo ones-row matmul
"""

import os
import sys

import numpy as np

try:
    import concourse  # noqa: F401
except ImportError:
    sys.path.insert(0, "/opt/trn_rl_repo")

import ml_dtypes
from concourse import bass, mybir
from concourse.bass_utils import run_bass_kernel_spmd
from concourse.tile import TileContext

B, N, F, H, D = 16, 512, 512, 8, 64
NCORES = 8
BPC = B // NCORES  # batches per core
IC = N // 128      # 128-row chunks per sequence

f32 = mybir.dt.float32
f32r = mybir.dt.float32r
bf16 = mybir.dt.bfloat16
AF = mybir.ActivationFunctionType
OP = mybir.AluOpType
AX = mybir.AxisListType


def _split_multiwaits(nc: bass.Bass) -> bass.Bass:
    """Walrus codegen only accepts one sync-wait per ISA instruction; hoist
    extra waits into single-wait NoOps on the same engine right before."""
    for fn in nc.m.functions:
        for blk in fn.blocks:
            new = []
            for inst in blk.instructions:
                si = getattr(inst, "sync_info", None)
                ow = list(si.on_wait) if (si is not None and si.on_wait) else []
                if len(ow) > 1:
                    for j, w in enumerate(ow[:-1]):
                        new.append(mybir.InstNoOp(
                            name=f"{inst.name}-wsplit{j}",
                            engine=inst.engine,
                            ins=[], outs=[],
                            sync_info=mybir.SyncInfo(on_wait=[w], on_update=[]),
                        ))
                    si.on_wait = ow[-1:]
                    inst.sync_info = si
                new.append(inst)
            blk.instructions[:] = new
    return nc


def build_nc(split: bool = True) -> bass.Bass:
    nc = bass.Bass()

    ndt_d = nc.declare_dram_parameter("ndt", [BPC, F, N], f32r, isOutput=False)
    # bias with mask pre-merged (-1e30), layout [b, pair, ic, p, u, j] bf16
    bias_d = nc.declare_dram_parameter("bias", [BPC, 4, IC, 128, 2, N], bf16,
                                       isOutput=False)
    # attn_mul, same layout
    mul_d = nc.declare_dram_parameter("mul", [BPC, 4, IC, 128, 2, N], bf16,
                                      isOutput=False)
    wq_d = nc.declare_dram_parameter("wq", [F, F], f32r, isOutput=False)  # x8
    wk_d = nc.declare_dram_parameter("wk", [F, F], f32r, isOutput=False)
    wv_d = nc.declare_dram_parameter("wv", [F, F], f32r, isOutput=False)
    wo_d = nc.declare_dram_parameter("wo", [F, F], bf16, isOutput=False)
    # [bv | bo | ones] as one row, f32r
    brow_d = nc.declare_dram_parameter("brow", [1, 3 * F], f32r, isOutput=False)
    # bq*8 / bk as per-partition columns, 4 ft chunks each
    bqk_d = nc.declare_dram_parameter("bqk", [128, 8], f32, isOutput=False)
    idb_d = nc.declare_dram_parameter("idb", [128, 128], bf16, isOutput=False)
    out_d = nc.declare_dram_parameter("out", [BPC, N, F], f32, isOutput=True)

    with (
        TileContext(nc) as tc,
        tc.tile_pool(name="cpool", bufs=1) as cpool,
        tc.tile_pool(name="bpool", bufs=1) as bpool,
        tc.tile_pool(name="spool", bufs=2) as spool,
        tc.tile_pool(name="wpool", bufs=2) as wpool,
        tc.tile_pool(name="dpool", bufs=1, space="DRAM") as dpool,
        tc.tile_pool(name="ps_s", bufs=2, space="PSUM") as ps_s,
        tc.tile_pool(name="ps_mm", bufs=2, space="PSUM") as ps_mm,
        tc.tile_pool(name="ps_o", bufs=2, space="PSUM") as ps_o,
    ):
        # ---- constants / weights ----
        wq_sb = cpool.tile([128, 4 * F], f32r, name="wq_sb")
        wk_sb = cpool.tile([128, 4 * F], f32r, name="wk_sb")
        wv_sb = cpool.tile([128, 4 * F], f32r, name="wv_sb")
        wo_sb = cpool.tile([128, 4 * F], bf16, name="wo_sb")
        for cc in range(4):
            nc.scalar.dma_start(wq_sb[:, cc * F:(cc + 1) * F], wq_d[cc * 128:(cc + 1) * 128, :])
            nc.scalar.dma_start(wk_sb[:, cc * F:(cc + 1) * F], wk_d[cc * 128:(cc + 1) * 128, :])
        brow_sb = cpool.tile([1, 3 * F], f32r, name="brow_sb")
        nc.sync.dma_start(brow_sb[:], brow_d[:, :])
        bv_row = brow_sb[:, 0:F]            # rhs for V bias
        bo_row = brow_sb[:, F:2 * F]
        ones_row = brow_sb[:, 2 * F:3 * F]  # [1, 512] of ones
        bqk_sb = cpool.tile([128, 8], f32, name="bqk_sb")
        nc.sync.dma_start(bqk_sb[:], bqk_d[:, :])
        idb_sb = cpool.tile([128, 128], bf16, name="idb_sb")
        nc.sync.dma_start(idb_sb[:], idb_d[:, :])

        def batch_inputs(b):
            st = {"b": b}
            ndt_sb = bpool.tile([128, 4 * N], f32r, name="ndt_sb", tag="ndt", bufs=2)
            for cc in range(4):
                nc.scalar.dma_start(ndt_sb[:, cc * N:(cc + 1) * N],
                                  ndt_d[b, cc * 128:(cc + 1) * 128, :])
            st["ndt"] = ndt_sb
            st["v"] = bpool.tile([128, 4 * F], bf16, name="v_sb", tag="v", bufs=2)
            st["ot"] = bpool.tile([128, 4 * N], bf16, name="ot_sb", tag="ot", bufs=2)
            if b == 0:
                for cc in range(4):
                    nc.scalar.dma_start(wv_sb[:, cc * F:(cc + 1) * F],
                                      wv_d[cc * 128:(cc + 1) * 128, :])
                    nc.scalar.dma_start(wo_sb[:, cc * F:(cc + 1) * F],
                                      wo_d[cc * 128:(cc + 1) * 128, :])
            return st

        def qk_proj(st, t):
            ndt_sb = st["ndt"]
            psq = ps_mm.tile([128, 512], f32, name="ps_q", tag="mm")
            for cc in range(4):
                nc.tensor.matmul(
                    psq[:],
                    lhsT=wq_sb[:, cc * F + t * 128: cc * F + t * 128 + 128],
                    rhs=ndt_sb[:, cc * N:(cc + 1) * N],
                    start=(cc == 0), stop=(cc == 3),
                )
            qt = wpool.tile([128, 512], f32r, name="qt", tag="qt", bufs=3)
            nc.scalar.activation(qt[:], psq[:], AF.Identity,
                                 bias=bqk_sb[:, t:t + 1], scale=1.0)
            psk = ps_mm.tile([128, 512], f32, name="ps_k", tag="mm")
            for cc in range(4):
                nc.tensor.matmul(
                    psk[:],
                    lhsT=wk_sb[:, cc * F + t * 128: cc * F + t * 128 + 128],
                    rhs=ndt_sb[:, cc * N:(cc + 1) * N],
                    start=(cc == 0), stop=(cc == 3),
                )
            kt = wpool.tile([128, 512], f32r, name="kt", tag="kt", bufs=3)
            nc.scalar.activation(kt[:], psk[:], AF.Identity,
                                 bias=bqk_sb[:, 4 + t:5 + t], scale=1.0)
            st["qt"], st["kt"] = qt, kt

        def v_proj(st, jc):
            ndt_sb, v_sb = st["ndt"], st["v"]
            psv = ps_mm.tile([128, 512], f32, name="ps_v", tag="mm")
            for cc in range(4):
                nc.tensor.matmul(
                    psv[:],
                    lhsT=ndt_sb[:, cc * N + jc * 128: cc * N + jc * 128 + 128],
                    rhs=wv_sb[:, cc * F:(cc + 1) * F],
                    start=(cc == 0), stop=False,
                )
            nc.tensor.matmul(psv[:], lhsT=ones_row[:, 0:128], rhs=bv_row,
                             start=False, stop=True)
            nc.scalar.copy(v_sb[:, jc * F:(jc + 1) * F], psv[:])

        def prefetch_pair(b, t):
            """biasm/mul chunk DMAs for pair (b, t), issued one stage ahead."""
            tiles = []
            for ic in range(IC):
                biasic = spool.tile([128, 2, N], bf16, name="biasic",
                                    tag="biasic", bufs=9)
                nc.scalar.dma_start(biasic[:], bias_d[b, t, ic])
                mulic = spool.tile([128, 2, N], bf16, name="mulic",
                                   tag="mulic", bufs=9)
                nc.scalar.dma_start(mulic[:], mul_d[b, t, ic])
                tiles.append((biasic, mulic))
            return tiles

        def s_chunks(b, st, t, scr, tiles, ics, pend):
            """Scores + softmax for chunks `ics` of head pair t; pm chunks go
            to DRAM scr. Normalize is pipelined one chunk behind the exp."""
            qt, kt = st["qt"], st["kt"]

            def flush_pend():
                if pend[0] is None:
                    return
                ic, e_u, z2, mulic = pend[0]
                rz2 = wpool.tile([128, 2], f32, name="rz2", tag="rz2", bufs=4)
                nc.vector.reciprocal(rz2[:], z2[:])
                for u in range(2):
                    pm0 = wpool.tile([128, 512], bf16, name="pm0",
                                     tag="pm0", bufs=4)
                    nc.gpsimd.tensor_tensor(pm0[:], e_u[u][:], mulic[:, u, :],
                                            OP.mult)
                    pm = wpool.tile([128, 512], bf16, name="pm", tag="pm", bufs=6)
                    nc.vector.tensor_scalar(pm[:], pm0[:], rz2[:, u:u + 1],
                                            None, OP.mult)
                    nc.sync.dma_start(
                        scr[u, ic * 128:(ic + 1) * 128, :], pm[:])
                pend[0] = None

            for ic in ics:
                biasic, mulic = tiles[ic]
                z2 = wpool.tile([128, 2], f32, name="z2", tag="z2", bufs=4)
                e_u = []
                for u in range(2):
                    sp = ps_s.tile([128, N], f32, name="sp", tag="sp", bufs=4)
                    nc.tensor.matmul(
                        sp[:],
                        lhsT=qt[u * 64:u * 64 + 64, ic * 128:ic * 128 + 128],
                        rhs=kt[u * 64:u * 64 + 64, :],
                        start=True, stop=False,
                    )
                    nc.tensor.matmul(
                        sp[:], lhsT=idb_sb[:], rhs=biasic[:, u, :],
                        start=False, stop=True,
                    )
                    negmax = wpool.tile([128, 1], f32, name="negmax",
                                        tag="negmax", bufs=8)
                    nc.vector.tensor_reduce(negmax[:], sp[:], AX.X, OP.max,
                                            negate=True)
                    e_sb = wpool.tile([128, N], bf16, name="e_sb", tag="e",
                                      bufs=6)
                    nc.scalar.activation(e_sb[:], sp[:], AF.Exp,
                                         bias=negmax[:], scale=1.0,
                                         accum_out=z2[:, u:u + 1])
                    e_u.append(e_sb)
                flush_pend()
                pend[0] = (ic, e_u, z2, mulic)
            if not ics:
                flush_pend()

        def tav_stage(st, t, scr):
            """DMA-transpose pm from DRAM and run AV matmuls for head pair t."""
            v_sb, ot_sb = st["v"], st["ot"]
            av = ps_o.tile([128, N], f32, name="av_ps", tag="av")
            pmts = {}
            for jc in range(4):
                for u in range(2):
                    pmt = spool.tile([128, N], bf16, name="pmt", tag="pmt",
                                     bufs=10)
                    nc.sync.dma_start_transpose(
                        pmt[:], scr[u, :, jc * 128:(jc + 1) * 128])
                    pmts[jc, u] = pmt
            for jc in range(4):
                for u in range(2):
                    h = 2 * t + u
                    nc.tensor.matmul(
                        av[u * 64:(u + 1) * 64, :],
                        lhsT=v_sb[:, jc * F + h * 64: jc * F + h * 64 + 64],
                        rhs=pmts[jc, u][:],
                        start=(jc == 0), stop=(jc == 3),
                        tile_position=(0, u * 64),
                    )
            return av

        def ot_copy(st, t, av):
            nc.vector.tensor_copy(st["ot"][:, t * N:(t + 1) * N], av[:])

        def final_stage(b, st):
            ot_sb = st["ot"]
            for icq in range(IC):
                f_ps = ps_mm.tile([128, N], f32, name="f_ps", tag="mm")
                for cc in range(4):
                    nc.tensor.matmul(
                        f_ps[:],
                        lhsT=ot_sb[:, cc * N + icq * 128: cc * N + icq * 128 + 128],
                        rhs=wo_sb[:, cc * F:(cc + 1) * F],
                        start=(cc == 0), stop=False,
                    )
                nc.tensor.matmul(f_ps[:], lhsT=ones_row[:, 0:128], rhs=bo_row,
                                 start=False, stop=True)
                out_sb = wpool.tile([128, N], f32, name="out_sb",
                                    tag="out", bufs=2)
                nc.scalar.copy(out_sb[:], f_ps[:])
                nc.sync.dma_start(out_d[b, icq * 128:(icq + 1) * 128, :], out_sb[:])

        # ---- cross-batch pair pipeline: 8 head-pair stages (2 batches x 4);
        # TAV of pair k-1 runs between the score chunk halves of pair k; the
        # cross-stage ot copy sits at the stage end so DVE never blocks ----
        sts = {0: batch_inputs(0)}
        scrs = {}
        avs = {}
        tiles = {0: prefetch_pair(0, 0)}
        for k in range(10):
            if k < 8:
                b, t = divmod(k, 4)
                st = sts[b]
                qk_proj(st, t)
                scrs[k] = dpool.tile([2, N, N], bf16, name="scr",
                                     tag="scr", bufs=3)
                pend = [None]
                s_chunks(b, st, t, scrs[k], tiles[k], [0, 1], pend)
                if k + 1 < 8:
                    bn, tn = divmod(k + 1, 4)
                    tiles[k + 1] = prefetch_pair(bn, tn)
            if 1 <= k <= 8:
                bb, tt = divmod(k - 1, 4)
                avs[k - 1] = tav_stage(sts[bb], tt, scrs[k - 1])
            if k < 8:
                s_chunks(b, st, t, scrs[k], tiles[k], [2, 3], pend)
            if k == 5:
                final_stage(0, sts[0])
            if k == 9:
                final_stage(1, sts[1])
            if k < 8:
                if t == 0:
                    for jc in range(4):
                        v_proj(st, jc)
                s_chunks(b, st, t, scrs[k], tiles[k], [], pend)
            if 1 <= k <= 8:
                ot_copy(sts[bb], tt, avs[k - 1])
            if k == 2:
                sts[1] = batch_inputs(1)

    mybir.codegen_inst_isa_subclasses(nc)
    return _split_multiwaits(nc) if split else nc


def make_in_maps(inputs: dict) -> list:
    inp = {k: np.asarray(v) for k, v in inputs.items()}
    ident = np.eye(128, dtype=np.float32)
    brow = np.concatenate([
        inp["bv"].astype(np.float32),
        inp["bo"].astype(np.float32),
        np.ones(F, np.float32),
    ]).reshape(1, 3 * F)
    bqk = np.concatenate([
        (inp["bq"].astype(np.float32) * 8.0).reshape(4, 128).T,
        inp["bk"].astype(np.float32).reshape(4, 128).T,
    ], axis=1)
    shared = {
        "wq": np.ascontiguousarray(inp["Wq"].astype(np.float32) * 8.0),
        "wk": np.ascontiguousarray(inp["Wk"].astype(np.float32)),
        "wv": np.ascontiguousarray(inp["Wv"].astype(np.float32)),
        "wo": np.ascontiguousarray(inp["Wo"].astype(ml_dtypes.bfloat16)),
        "brow": brow,
        "bqk": np.ascontiguousarray(bqk),
        "idb": ident.astype(ml_dtypes.bfloat16),
    }
    # [B, i, j, h] -> [B, pair, ic, p, u, j]
    def to_chunks(x):
        x = x.transpose(0, 3, 1, 2)                      # [B, h, i, j]
        x = x.reshape(B, 4, 2, IC, 128, N)               # [B, t, u, ic, p, j]
        return np.ascontiguousarray(x.transpose(0, 1, 3, 4, 2, 5))

    biasm = np.where(inp["attn_mask"][..., None] != 0, np.float32(-1e30),
                     inp["attn_bias"].astype(np.float32))
    biasm = to_chunks(biasm).astype(ml_dtypes.bfloat16)
    mul_t = to_chunks(inp["attn_mul"].astype(np.float32)).astype(ml_dtypes.bfloat16)
    ndt = np.ascontiguousarray(
        inp["ndata"].astype(np.float32).transpose(0, 2, 1))
    in_maps = []
    for c in range(NCORES):
        sl = slice(c * BPC, (c + 1) * BPC)
        m = dict(shared)
        m["ndt"] = np.ascontiguousarray(ndt[sl])
        m["bias"] = np.ascontiguousarray(biasm[sl])
        m["mul"] = np.ascontiguousarray(mul_t[sl])
        in_maps.append(m)
    return in_maps


def kernel(**inputs) -> np.ndarray:
    nc = build_nc()
    in_maps = make_in_maps(inputs)
    res = run_bass_kernel_spmd(nc, in_maps, list(range(NCORES)))
    out = np.concatenate([np.asarray(res.results[c]["out"]) for c in range(NCORES)],
                         axis=0)
    return out.astype(np.float32)


if __name__ == "__main__":
    nc = build_nc()
    print("built ok")
